# revision 1
# baseline (speedup 1.0000x reference)
"""Trainium2 Bass kernel for nn_CIFARViT: 8-layer ViT with a per-head
mini-transformer over attention maps. Data-parallel: one batch element
per NeuronCore (8 cores), full inputs in / full outputs out.

Per-core layout conventions:
  - residual stream h: token-major (TM) f32, fold [128, 2*768]
    (h[l, c] -> tile[l % 128, (l // 128)*768 + c])
  - matmul operands bf16; feature-major (FM) tensors fold the feature
    dim onto partitions in 128-row chunks.
  - weights: host-pretransposed W^T bf16 images, streamed per layer.
  - LayerNorm: free-dim stats in TM; the (x-mu)*rstd and the TM->FM
    transpose are fused into one PE matmul with rhs = diag(rstd);
    gamma/beta applied on the PSUM->SBUF copy (per-partition scalars).
  - softmax(last axis): free-dim stats. softmax(axis=-2) (wn) is kept
    unnormalized (X = exp(pre)); column sums Z via ones-matmul; 1/Z и
    Z*bias folded into the consuming einsum (per-partition scale after
    transposed contraction).
"""
import sys

sys.path.insert(0, "/opt/trn_rl_repo")

import numpy as np
import ml_dtypes

import concourse.bass as bass
import concourse.mybir as mybir
import concourse.tile as tile
from concourse import bacc
from concourse.bass_utils import run_bass_kernel_spmd

BF = ml_dtypes.bfloat16
F32 = np.float32
AF = mybir.ActivationFunctionType
ALU = mybir.AluOpType
bf = mybir.dt.bfloat16
f32 = mybir.dt.float32

H = 8
HD = 96
C = 768
L = 256
D_LAYERS = 8
F = 3072

N_CORES = 8
X_AXIS = mybir.AxisListType.X
import os
DBG_LAYERS = int(os.environ.get("KLAYERS", "8"))
DBG_TAP = os.environ.get("KTAP", "") == "1"
DBG_EPI = os.environ.get("KEPI", "1") == "1"



def _fold(wt):
    """[R, Cc] with R = 128*T -> [128, T*Cc] partition fold."""
    R, Cc = wt.shape
    T = R // 128
    return np.ascontiguousarray(
        wt.reshape(T, 128, Cc).transpose(1, 0, 2).reshape(128, T * Cc)
    )


def _foldv(v):
    T = v.shape[0] // 128
    return np.ascontiguousarray(v.reshape(T, 128).T)


def marshal(inputs):
    inp = {k: np.asarray(v) for k, v in inputs.items()}
    sh = {}
    qk_l, v_l, proj_l, w1_l, w2_l, lnp_l, b1_l, rowb_l = ([] for _ in range(8))
    for i in range(D_LAYERS):
        qkvT = inp["qkv_w"][i].T.astype(BF)  # [768, 2304]
        qkf = _fold(qkvT)  # [128, 6*2304]
        img = np.zeros((128, 4 * 2304), dtype=BF)
        for hp in range(4):
            for cb in range(6):
                for hh in range(2):
                    h = hp * 2 + hh
                    base = hp * 2304 + cb * 384 + hh * 192
                    img[:, base:base + 96] = \
                        qkf[:, cb * 2304 + 96 * h: cb * 2304 + 96 * h + 96]
                    img[:, base + 96:base + 192] = \
                        qkf[:, cb * 2304 + 768 + 96 * h: cb * 2304 + 768 + 96 * h + 96]
        qk_l.append(img)
        v_l.append(_fold(np.ascontiguousarray(qkvT[:, 1536:2304])))
        proj_l.append(_fold(inp["proj_w"][i].T.astype(BF)))
        w1T = inp["mlp_w1"][i].T.astype(BF)  # [768, 3072]
        w1_l.append(np.ascontiguousarray(
            w1T.reshape(6, 128, 24, 128).transpose(1, 2, 0, 3).reshape(128, 24 * 768)))
        w2_l.append(_fold(inp["mlp_w2"][i].T.astype(BF)))  # [128, 24*768]
        lnp_l.append(np.concatenate(
            [_foldv(inp[k][i].astype(F32))
             for k in ("ln1_g", "ln1_b", "ln2_g", "ln2_b")], axis=1))
        b1_l.append(_foldv(inp["mlp_b1"][i].astype(F32)))
        rb = np.zeros((2, 2 * C), dtype=BF)
        rb[0, :C] = inp["proj_b"][i].astype(BF)
        rb[0, C:] = inp["mlp_b2"][i].astype(BF)
        rowb_l.append(rb)
    sh["qk_img"] = np.stack(qk_l)
    sh["v_img"] = np.stack(v_l)
    sh["projT"] = np.stack(proj_l)
    sh["w1T"] = np.stack(w1_l)
    sh["w2T"] = np.stack(w2_l)
    sh["lnp"] = np.stack(lnp_l)
    sh["b1f"] = np.stack(b1_l)
    sh["rowb"] = np.stack(rowb_l)

    sh["mqkvT"] = _fold(inp["m_qkv_w"].T.astype(BF))   # [128, 2*768]
    sh["mprojT"] = _fold(inp["m_proj_w"].T.astype(BF))  # [128, 2*256]
    sh["mw1T"] = _fold(inp["m_mlp_w1"].T.astype(BF))   # [128, 2*1024]
    sh["mw2T"] = _fold(inp["m_mlp_w2"].T.astype(BF))   # [128, 8*256]
    sh["mlnp"] = np.concatenate(
        [_foldv(inp[k].astype(F32))
         for k in ("m_ln1_g", "m_ln1_b", "m_ln2_g", "m_ln2_b")], axis=1)
    sh["mb1f"] = _foldv(inp["m_mlp_b1"].astype(F32))   # [128, 8]
    mb1r_ = np.zeros((2, 1024), dtype=BF)
    mb1r_[0, :] = inp["m_mlp_b1"].astype(BF)
    sh["mb1r"] = mb1r_
    onesr_ = np.zeros((2, 256), dtype=BF)
    onesr_[0, :] = 1.0
    sh["onesr"] = onesr_
    mrb_ = np.zeros((2, 512), dtype=BF)
    mrb_[0, :256] = inp["m_proj_b"].astype(BF)
    mrb_[0, 256:] = inp["m_mlp_b2"].astype(BF)
    sh["mrowb"] = mrb_

    sh["pwT"] = np.ascontiguousarray(inp["patch_w"].reshape(C, 12).T.astype(BF))
    pos = inp["pos_emb"][0].astype(F32) + inp["patch_b"][None, :].astype(F32)
    sh["pos"] = _fold(pos)  # [128, 2*768]
    sh["normgb"] = np.concatenate(
        [_foldv(inp["norm_g"].astype(F32)), _foldv(inp["norm_b"].astype(F32))],
        axis=1)  # [128, 12]
    sh["headwT"] = _fold(inp["head_w"].T.astype(F32))  # [128, 6*10]
    sh["headb"] = inp["head_b"].astype(F32).reshape(10, 1)
    sh["ident"] = np.eye(128, dtype=BF)
    sh["onescol"] = np.ones((128, 1), dtype=BF)
    sh["ones2"] = np.ones((2, 128), dtype=BF)
    e0_ = np.zeros((2, 1), dtype=BF); e0_[0, 0] = 1.0
    sh["e0"] = e0_
    sh["onescolf"] = np.ones((128, 1), dtype=F32)

    x = inp["x"].astype(F32)
    per_core = []
    for b in range(N_CORES):
        pt = (x[b].reshape(3, 16, 2, 16, 2).transpose(0, 2, 4, 1, 3)
              .reshape(12, 256).astype(BF))
        m = dict(sh)
        m["patchesT"] = np.ascontiguousarray(pt)
        per_core.append(m)
    return per_core


DT_MAP = {np.dtype(BF): bf, np.dtype(np.float32): f32}


def build(in_map):
    nc = bacc.Bacc("TRN2", target_bir_lowering=False, debug=False,
                   num_devices=N_CORES)
    dram = {k: nc.dram_tensor(k, v.shape, DT_MAP[v.dtype], kind="ExternalInput")
            for k, v in in_map.items()}
    out_d = nc.dram_tensor("out", (10, 1), f32, kind="ExternalOutput")
    dbg_d = nc.dram_tensor("dbg", (128, 2 * C), f32,
                           kind="ExternalOutput") if DBG_TAP else None
    with tile.TileContext(nc) as tc:
        _body(nc, tc, dram, out_d, dbg_d)
    nc.compile()
    return nc


def _body(nc, tc, dram, out_d, dbg_d=None):
    import contextlib
    ctx = contextlib.ExitStack()
    with ctx:
        P = lambda name, bufs=1, space="SBUF": ctx.enter_context(
            tc.tile_pool(name=name, bufs=bufs, space=space))
        cpool = P("const")

        def cload(name):
            arr = dram[name]
            t = cpool.tile(list(arr.shape), arr.dtype, tag=name, name=name)
            nc.sync.dma_start(t[:], arr[:])
            return t

        mqkvT = cload("mqkvT")
        mprojT = cload("mprojT")
        mw1T = cload("mw1T")
        mw2T = cload("mw2T")
        mlnp = cload("mlnp")
        mb1f = cload("mb1f")
        mb1r = cload("mb1r")
        onesr = cload("onesr")
        mrowb = cload("mrowb")
        ident = cload("ident")
        onescol = cload("onescol")
        ones2 = cload("ones2")
        e0 = cload("e0")
        normgb = cload("normgb")
        headwT = cload("headwT")
        headb = cload("headb")
        pwT = cload("pwT")
        patchesT = cload("patchesT")

        NDIAG = 3
        dtiles = []
        for j in range(NDIAG):
            t = cpool.tile([128, 256], bf, tag=f"diag{j}", name=f"diag{j}")
            nc.gpsimd.memset(t[:], 0.0)
            dtiles.append(t)
        dctr = [0]

        eps6 = cpool.tile([128, 1], f32, tag="eps6", name="eps6")
        nc.gpsimd.memset(eps6[:], 1e-6)
        eps5 = cpool.tile([128, 1], f32, tag="eps5", name="eps5")
        nc.gpsimd.memset(eps5[:], 1e-5)
        u32 = mybir.dt.uint32
        magic_t = cpool.tile([128, 32], u32, tag="magic", name="magic")
        nc.gpsimd.memset(magic_t[:], 0x5f3759df)

        RSQ_ACT = os.environ.get("KRSQ", "") == "act"

        def rsqrt_cols(src_ap, w, eps):
            """[128, w] f32 -> 1/sqrt(src+eps); int-seed + 2 Newton (DVE only)."""
            if RSQ_ACT:
                sdt = rsp.tile([128, 32], f32, tag="rsq_sd", name="rsq_sd")
                if eps:
                    epst = eps6 if eps == 1e-6 else eps5
                    nc.scalar.activation(sdt[:, 0:w], src_ap, AF.Sqrt,
                                         bias=epst[:])
                else:
                    nc.scalar.activation(sdt[:, 0:w], src_ap, AF.Sqrt)
                outt = rsp.tile([128, 32], f32, tag="rsq_oo", name="rsq_oo")
                nc.vector.reciprocal(outt[:, 0:w], sdt[:, 0:w])
                return outt[:, 0:w]
            if eps:
                v = rsp.tile([128, 32], f32, tag="rsq_v", name="rsq_v")
                nc.vector.tensor_scalar_add(v[:, 0:w], src_ap, eps)
                vap = v[:, 0:w]
            else:
                vap = src_ap
            shv = rsp.tile([128, 32], u32, tag="rsq_s", name="rsq_s")
            nc.vector.tensor_scalar(shv[:, 0:w], vap.bitcast(u32), 1, None,
                                    ALU.logical_shift_right)
            y0 = rsp.tile([128, 32], u32, tag="rsq_y", name="rsq_y")
            nc.vector.tensor_tensor(y0[:, 0:w], magic_t[:, 0:w], shv[:, 0:w],
                                    ALU.subtract)
            cur = y0[:, 0:w].bitcast(f32)
            for it in range(2):
                t1 = rsp.tile([128, 32], f32, tag=f"rsq_t{it}", name="rsq_t")
                nc.vector.tensor_tensor(t1[:, 0:w], cur, cur, ALU.mult)
                nc.vector.tensor_tensor(t1[:, 0:w], t1[:, 0:w], vap, ALU.mult)
                nc.vector.tensor_scalar(t1[:, 0:w], t1[:, 0:w], -0.5, 1.5,
                                        ALU.mult, ALU.add)
                nxt = rsp.tile([128, 32], f32, tag=f"rsq_o{it}", name="rsq_o")
                nc.vector.tensor_tensor(nxt[:, 0:w], cur, t1[:, 0:w], ALU.mult)
                cur = nxt[:, 0:w]
            return cur

        def build_diag(s0, s1):
            d = dtiles[dctr[0] % NDIAG]
            dctr[0] += 1
            nc.vector.tensor_scalar_mul(d[:, 0:128], ident[:], s0)
            nc.vector.tensor_scalar_mul(d[:, 128:256], ident[:], s1)
            return d

        hpool = P("h", bufs=3)
        ps = P("ps", bufs=4, space="PSUM")
        psy = P("psy", bufs=4, space="PSUM")
        stp = P("st", bufs=24)
        wqk = P("wqk", bufs=2)
        wv = P("wv", bufs=1)
        wproj = P("wproj", bufs=1)
        ww1 = P("ww1", bufs=2)
        ww2 = P("ww2", bufs=1)
        hlnp = P("hln", bufs=1)
        qkp = P("qkt", bufs=1)
        vtp = P("vt", bufs=1)
        ebp = P("eb", bufs=1)
        atp = P("at", bufs=1)
        ybp = P("yb", bufs=1)
        scr = P("scr", bufs=2)
        wnp = P("wn", bufs=8)
        mqp = P("mq", bufs=8)
        mvp = P("mvp", bufs=8)
        emp = P("emp", bufs=8)
        omp = P("omp", bufs=8)
        hmp = P("hm", bufs=8)
        rsp = P("rs", bufs=2)
        rbp = P("rb", bufs=1)
        y1p = P("y1", bufs=2)
        zp = P("zp", bufs=2)
        zwp = P("zw", bufs=1)

        def st():
            return stp.tile([128, 1], f32, tag="st", name="st")

        def mm(out, lhsT, rhs, start, stop):
            nc.tensor.matmul(out, lhsT, rhs, start=start, stop=stop)

        # ---------------- prologue: patch embed ----------------
        pos_t = hpool.tile([128, 2 * C], f32, tag="h", name="pos")
        h_t = hpool.tile([128, 2 * C], f32, tag="h", name="h")
        nc.sync.dma_start(pos_t[:], dram["pos"][:])
        for lc in range(2):
            for n0, nw in ((0, 512), (512, 256)):
                p = ps.tile([128, 512], f32, tag="ps", name="ps")
                mm(p[:, 0:nw], patchesT[0:12, lc * 128:lc * 128 + 128],
                   pwT[0:12, n0:n0 + nw], True, True)
                nc.vector.tensor_add(
                    h_t[:, lc * C + n0:lc * C + n0 + nw], p[:, 0:nw],
                    pos_t[:, lc * C + n0:lc * C + n0 + nw])

        # ---------------- layers ----------------
        for li in range(DBG_LAYERS):
            lnp_t = scr.tile([128, 24], f32, tag="lnp", name="lnp")
            nc.sync.dma_start(lnp_t[:], dram["lnp"][li])
            b1f_t = scr.tile([128, 24], f32, tag="b1f", name="b1f")
            nc.sync.dma_start(b1f_t[:], dram["b1f"][li])
            rowb_t = rbp.tile([2, 2 * C], bf, tag="rowb", name="rowb")
            nc.sync.dma_start(rowb_t[:], dram["rowb"][li])
            vt_w = wv.tile([128, 6 * C], bf, tag="wv", name="wv")
            nc.sync.dma_start(vt_w[:], dram["v_img"][li])
            projT = wproj.tile([128, 6 * C], bf, tag="wproj", name="wproj")
            nc.sync.dma_start(projT[:], dram["projT"][li])

            def ln_transpose_outer(src, g_col, b_col, epsv, tag, pool):
                """TM f32 [128, 2*768] -> LN'd FM bf16 [128, 6*256]."""
                dst = pool.tile([128, 6 * 256], bf, tag=tag)
                subs = rbp.tile([128, 2 * C], bf, tag="lnsub", name="lnsub")
                mv4 = zwp.tile([128, 4], f32, tag="mv4", name="mv4")
                for lc in range(2):
                    bns = stp.tile([128, 12], f32, tag="bns12", name="bns12")
                    nc.vector.bn_stats(
                        bns[:, 0:6], src[:, lc * C:lc * C + 384])
                    nc.vector.bn_stats(
                        bns[:, 6:12], src[:, lc * C + 384:(lc + 1) * C])
                    nc.vector.bn_aggr(mv4[:, lc * 2:lc * 2 + 2], bns[:])
                rs4 = rsqrt_cols(mv4[:], 4, epsv)
                for lc in range(2):
                    nc.vector.tensor_scalar_sub(
                        subs[:, lc * C:(lc + 1) * C],
                        src[:, lc * C:(lc + 1) * C],
                        mv4[:, lc * 2:lc * 2 + 1])
                d = build_diag(rs4[:, 1:2], rs4[:, 3:4])
                for cb in range(6):
                    p = ps.tile([128, 512], f32, tag="ps", name="ps")
                    for lc in range(2):
                        mm(p[:, lc * 128:lc * 128 + 128],
                           subs[:, lc * C + cb * 128:lc * C + cb * 128 + 128],
                           d[:, lc * 128:lc * 128 + 128], True, True)
                    nc.vector.tensor_scalar(
                        dst[:, cb * 256:(cb + 1) * 256], p[:, 0:256],
                        lnp_t[:, g_col + cb:g_col + cb + 1],
                        lnp_t[:, b_col + cb:b_col + cb + 1],
                        ALU.mult, ALU.add)
                return dst

            hln = ln_transpose_outer(h_t, 0, 6, 1e-6, "hln", hlnp)

            # ---- qkv: q^T|k^T per head [96, 512]; v token-major ----
            qk_bf = qkp.tile([128, H * 512], bf, tag="qk", name="qk")
            for hp in range(4):
                qkw = wqk.tile([128, 2304], bf, tag="wqk", name="wqk")
                nc.sync.dma_start(
                    qkw[:], dram["qk_img"][li, :, hp * 2304:(hp + 1) * 2304])
                for hh in range(2):
                    h = hp * 2 + hh
                    p = ps.tile([128, 512], f32, tag="ps", name="ps")
                    for half in range(2):
                        for cb in range(6):
                            lh = qkw[:, cb * 384 + hh * 192 + half * 96:
                                     cb * 384 + hh * 192 + half * 96 + 96]
                            mm(p[0:96, half * 256:half * 256 + 256], lh,
                               hln[:, cb * 256:(cb + 1) * 256],
                               cb == 0, cb == 5)
                    nc.scalar.copy(
                        qk_bf[0:96, h * 512:(h + 1) * 512], p[0:96, :])
            vt_bf = vtp.tile([128, 2 * C], bf, tag="vt", name="vt")
            for lc in range(2):
                for n0, nw in ((0, 512), (512, 256)):
                    p = ps.tile([128, 512], f32, tag="ps", name="ps")
                    for cb in range(6):
                        mm(p[:, 0:nw],
                           hln[:, cb * 256 + lc * 128:cb * 256 + lc * 128 + 128],
                           vt_w[:, cb * C + n0:cb * C + n0 + nw],
                           cb == 0, cb == 5)
                    nc.vector.tensor_copy(
                        vt_bf[:, lc * C + n0:lc * C + n0 + nw], p[:, 0:nw])

            # ---- S3: scores -> E = exp(s); Z, 1/Z (all heads) ----
            E_bf = ebp.tile([128, H * 512], bf, tag="eb", name="eb")
            aT_bf = atp.tile([128, H * 512], bf, tag="at", name="at")
            zt = zwp.tile([128, 16], f32, tag="zt", name="zt")
            izt = zwp.tile([128, 16], f32, tag="izt", name="izt")
            for h in range(H):
                p = ps.tile([128, 512], f32, tag="ps", name="ps")
                for lc in range(2):
                    mm(p[:, lc * 256:lc * 256 + 256],
                       qk_bf[0:96, h * 512 + lc * 128:h * 512 + lc * 128 + 128],
                       qk_bf[0:96, h * 512 + 256:h * 512 + 512], True, True)
                    nc.scalar.activation(
                        E_bf[:, h * 512 + lc * 256:h * 512 + lc * 256 + 256],
                        p[:, lc * 256:lc * 256 + 256], AF.Exp, scale=HD ** -0.5,
                        accum_out=zt[:, h * 2 + lc:h * 2 + lc + 1])
                nc.vector.reciprocal(izt[:, h * 2:h * 2 + 2],
                                     zt[:, h * 2:h * 2 + 2])
            # ---- S4: aT = (E/Z)^T (all heads) ----
            for h in range(H):
                d = build_diag(izt[:, h * 2:h * 2 + 1],
                               izt[:, h * 2 + 1:h * 2 + 2])
                pa = ps.tile([128, 512], f32, tag="ps", name="ps")
                for mc in range(2):
                    for lc in range(2):
                        mm(pa[:, mc * 256 + lc * 128:mc * 256 + lc * 128 + 128],
                           E_bf[:, h * 512 + lc * 256 + mc * 128:
                                h * 512 + lc * 256 + mc * 128 + 128],
                           d[:, lc * 128:lc * 128 + 128], True, True)
                nc.scalar.copy(
                    aT_bf[:, h * 512:(h + 1) * 512], pa[:])
            # ---- S5: attnV -> oT ----
            oT_bf = hlnp.tile([128, 6 * 256], bf, tag="ot", name="ot")
            for h in range(H):
                po = ps.tile([128, 512], f32, tag="ps", name="ps")
                for mc in range(2):
                    mm(po[0:96, 0:256],
                       vt_bf[:, mc * C + 96 * h:mc * C + 96 * h + 96],
                       aT_bf[:, h * 512 + mc * 256:h * 512 + mc * 256 + 256],
                       mc == 0, mc == 1)
                for k in range(3):
                    rr = 96 * h + 32 * k
                    cb, r0 = divmod(rr, 128)
                    nc.vector.tensor_copy(
                        oT_bf[r0:r0 + 32, cb * 256:(cb + 1) * 256],
                        po[32 * k:32 * k + 32, 0:256])

            # ---- S6: proj -> y (TM bf16) ----
            y_bf = ybp.tile([128, 2 * C], bf, tag="yb", name="yb")
            for lc in range(2):
                for n0, nw in ((0, 512), (512, 256)):
                    p = ps.tile([128, 512], f32, tag="ps", name="ps")
                    for cb in range(6):
                        mm(p[:, 0:nw],
                           oT_bf[:, cb * 256 + lc * 128:cb * 256 + lc * 128 + 128],
                           projT[:, cb * C + n0:cb * C + n0 + nw],
                           cb == 0, cb == 5)
                    nc.scalar.copy(
                        y_bf[:, lc * C + n0:lc * C + n0 + nw], p[:, 0:nw])

            # ---- S7: mini LN1 (batched stats over all heads) ----
            # LN of a = E/Z without normalizing: mean_a = 1/L exactly;
            # var_a = sumsq(E)*iz^2/L - 1/L^2. Centering const zc = Z/L.
            sqt = zwp.tile([128, 16], f32, tag="sqt", name="sqt")
            for h in range(H):
                for lc in range(2):
                    esl = E_bf[:, h * 512 + lc * 256:h * 512 + lc * 256 + 256]
                    sqs = scr.tile([128, 256], bf, tag="msq", name="msq")
                    nc.vector.scalar_tensor_tensor(
                        sqs[:], esl, 1.0, esl, ALU.mult, ALU.mult,
                        accum_out=sqt[:, h * 2 + lc:h * 2 + lc + 1])
            iz2t = zwp.tile([128, 16], f32, tag="iz2t", name="iz2t")
            nc.vector.tensor_tensor(iz2t[:], izt[:], izt[:], ALU.mult)
            nc.vector.tensor_tensor(iz2t[:], sqt[:], iz2t[:], ALU.mult)
            vart = zwp.tile([128, 16], f32, tag="vart", name="vart")
            nc.vector.tensor_scalar(
                vart[:], iz2t[:], 1.0 / L, 1.0 / (L * L) - 1e-6,
                ALU.mult, ALU.subtract)
            ra1 = rsqrt_cols(vart[:], 16, 0.0)
            rzt = zwp.tile([128, 16], f32, tag="rzt", name="rzt")
            nc.vector.tensor_tensor(rzt[:], ra1, izt[:], ALU.mult)
            zct = zwp.tile([128, 16], f32, tag="zct", name="zct")
            nc.vector.tensor_scalar_mul(zct[:], zt[:], 1.0 / L)
            wnl = []
            for h in range(H):
                esub = scr.tile([128, 512], bf, tag="esub", name="esub")
                for lc in range(2):
                    nc.vector.tensor_scalar_sub(
                        esub[:, lc * 256:(lc + 1) * 256],
                        E_bf[:, h * 512 + lc * 256:h * 512 + lc * 256 + 256],
                        zct[:, h * 2 + lc:h * 2 + lc + 1])
                d = build_diag(rzt[:, h * 2:h * 2 + 1],
                               rzt[:, h * 2 + 1:h * 2 + 2])
                pw_ = ps.tile([128, 512], f32, tag="ps", name="ps")
                for mc in range(2):
                    for lc in range(2):
                        mm(pw_[:, mc * 256 + lc * 128:mc * 256 + lc * 128 + 128],
                           esub[:, lc * 256 + mc * 128:lc * 256 + mc * 128 + 128],
                           d[:, lc * 128:lc * 128 + 128], True, True)
                wnl_h = wnp.tile([128, 512], bf, tag="wn", name="wn")
                for mc in range(2):
                    nc.vector.tensor_scalar(
                        wnl_h[:, mc * 256:mc * 256 + 256],
                        pw_[:, mc * 256:mc * 256 + 256],
                        mlnp[:, 0 + mc:1 + mc], mlnp[:, 2 + mc:3 + mc],
                        ALU.mult, ALU.add)
                wnl.append(wnl_h)

            # ---- S8: mini qkv (q|k interleaved per dc) + v ----
            mqk = []
            mv = []
            for h in range(H):
                mqk_h = mqp.tile([128, 1024], bf, tag="mqk", name="mqk")
                for dc in range(2):
                    p = ps.tile([128, 512], f32, tag="ps", name="ps")
                    for half in range(2):
                        for mc in range(2):
                            mm(p[:, half * 256:half * 256 + 256],
                               mqkvT[:, mc * 768 + half * 256 + dc * 128:
                                     mc * 768 + half * 256 + dc * 128 + 128],
                               wnl[h][:, mc * 256:mc * 256 + 256],
                               mc == 0, mc == 1)
                    nc.vector.tensor_copy(
                        mqk_h[:, dc * 512:dc * 512 + 512], p[:])
                mv_h = mvp.tile([128, 512], bf, tag="mv", name="mv")
                p2 = ps.tile([128, 512], f32, tag="ps", name="ps")
                for lc in range(2):
                    for mc in range(2):
                        mm(p2[:, lc * 256:lc * 256 + 256],
                           wnl[h][:, mc * 256 + lc * 128:mc * 256 + lc * 128 + 128],
                           mqkvT[:, mc * 768 + 512:mc * 768 + 768],
                           mc == 0, mc == 1)
                nc.scalar.copy(mv_h[:], p2[:])
                mqk.append(mqk_h)
                mv.append(mv_h)

            # ---- S9: mini scores -> Em = exp(s); Z, 1/Z ----
            mzt = zwp.tile([128, 16], f32, tag="mzt", name="mzt")
            mizt = zwp.tile([128, 16], f32, tag="mizt", name="mizt")
            Em = []
            for h in range(H):
                Em_h = emp.tile([128, 512], bf, tag="em", name="em")
                p = ps.tile([128, 512], f32, tag="ps", name="ps")
                for lc in range(2):
                    for dc in range(2):
                        mm(p[:, lc * 256:lc * 256 + 256],
                           mqk[h][:, dc * 512 + lc * 128:dc * 512 + lc * 128 + 128],
                           mqk[h][:, dc * 512 + 256:dc * 512 + 512],
                           dc == 0, dc == 1)
                    nc.scalar.activation(
                        Em_h[:, lc * 256:lc * 256 + 256],
                        p[:, lc * 256:lc * 256 + 256], AF.Exp, scale=0.0625,
                        accum_out=mzt[:, h * 2 + lc:h * 2 + lc + 1])
                nc.vector.reciprocal(mizt[:, h * 2:h * 2 + 2],
                                     mzt[:, h * 2:h * 2 + 2])
                Em.append(Em_h)

            # ---- S10: amT = (Em/Z)^T ----
            amT = []
            for h in range(H):
                d = build_diag(mizt[:, h * 2:h * 2 + 1],
                               mizt[:, h * 2 + 1:h * 2 + 2])
                pa = ps.tile([128, 512], f32, tag="ps", name="ps")
                for mc in range(2):
                    for lc in range(2):
                        mm(pa[:, mc * 256 + lc * 128:mc * 256 + lc * 128 + 128],
                           Em[h][:, lc * 256 + mc * 128:lc * 256 + mc * 128 + 128],
                           d[:, lc * 128:lc * 128 + 128], True, True)
                amT_h = wnp.tile([128, 512], bf, tag="wn", name="amt")
                nc.vector.tensor_copy(amT_h[:], pa[:])
                amT.append(amT_h)

            # ---- S11: mini attnV -> omT (FM) ----
            omT = []
            for h in range(H):
                po = ps.tile([128, 512], f32, tag="ps", name="ps")
                for dc in range(2):
                    for mc in range(2):
                        mm(po[:, dc * 256:dc * 256 + 256],
                           mv[h][:, mc * 256 + dc * 128:mc * 256 + dc * 128 + 128],
                           amT[h][:, mc * 256:mc * 256 + 256],
                           mc == 0, mc == 1)
                omT_h = omp.tile([128, 512], bf, tag="om", name="om")
                nc.vector.tensor_copy(omT_h[:], po[:])
                omT.append(omT_h)

            # ---- S12: hmini = omT.T@mprojT + aT.T@mprojT + pb (TM bf16) ----
            hmt = []
            for h in range(H):
                ph = ps.tile([128, 512], f32, tag="ps", name="ps")
                for lc in range(2):
                    for mc in range(2):
                        mm(ph[:, lc * 256:lc * 256 + 256],
                           omT[h][:, mc * 256 + lc * 128:mc * 256 + lc * 128 + 128],
                           mprojT[:, mc * 256:(mc + 1) * 256],
                           mc == 0, False)
                    for mc in range(2):
                        mm(ph[:, lc * 256:lc * 256 + 256],
                           aT_bf[:, h * 512 + mc * 256 + lc * 128:
                                 h * 512 + mc * 256 + lc * 128 + 128],
                           mprojT[:, mc * 256:(mc + 1) * 256],
                           False, False)
                    mm(ph[:, lc * 256:lc * 256 + 256], ones2[0:2, 0:128],
                       mrowb[0:2, 0:256], False, True)
                hm_h = hmp.tile([128, 512], bf, tag="hm", name="hm")
                nc.scalar.copy(hm_h[:], ph[:])
                hmt.append(hm_h)

            # ---- S13: mini LN2 (bn stats; batched rsqrt) ----
            mv2 = zwp.tile([128, 32], f32, tag="mv2", name="mv2")
            for h in range(H):
                for lc in range(2):
                    bns = stp.tile([128, 6], f32, tag="bns6", name="bns6")
                    nc.vector.bn_stats(
                        bns[:], hmt[h][:, lc * 256:(lc + 1) * 256])
                    cc = (h * 2 + lc) * 2
                    nc.vector.bn_aggr(mv2[:, cc:cc + 2], bns[:])
            rs2 = rsqrt_cols(mv2[:], 32, 1e-5)
            mhln = []
            for h in range(H):
                hsub = scr.tile([128, 512], bf, tag="esub", name="hsub")
                for lc in range(2):
                    cc = (h * 2 + lc) * 2
                    nc.vector.tensor_scalar_sub(
                        hsub[:, lc * 256:(lc + 1) * 256],
                        hmt[h][:, lc * 256:(lc + 1) * 256],
                        mv2[:, cc:cc + 1])
                d = build_diag(rs2[:, h * 4 + 1:h * 4 + 2],
                               rs2[:, h * 4 + 3:h * 4 + 4])
                ph2 = ps.tile([128, 512], f32, tag="ps", name="ps")
                for mc in range(2):
                    for lc in range(2):
                        mm(ph2[:, mc * 256 + lc * 128:mc * 256 + lc * 128 + 128],
                           hsub[:, lc * 256 + mc * 128:lc * 256 + mc * 128 + 128],
                           d[:, lc * 128:lc * 128 + 128], True, True)
                mhln_h = emp.tile([128, 512], bf, tag="em", name="mhln")
                for mc in range(2):
                    nc.vector.tensor_scalar(
                        mhln_h[:, mc * 256:mc * 256 + 256],
                        ph2[:, mc * 256:mc * 256 + 256],
                        mlnp[:, 4 + mc:5 + mc], mlnp[:, 6 + mc:7 + mc],
                        ALU.mult, ALU.add)
                mhln.append(mhln_h)

            # ---- S14/S15: mini MLP (gelu cluster) + wnpre into E slots ----
            for h in range(H):
                py = psy.tile([128, 512], f32, tag="psy", name="psy")
                for fp in range(4):
                    p1 = ps.tile([128, 512], f32, tag="ps", name="ps")
                    y1g = y1p.tile([128, 512], bf, tag="y1", name="y1")
                    for fo in range(2):
                        fc = fp * 2 + fo
                        for mc in range(2):
                            mm(p1[:, fo * 256:fo * 256 + 256],
                               mw1T[:, mc * 1024 + fc * 128:
                                    mc * 1024 + fc * 128 + 128],
                               mhln[h][:, mc * 256:mc * 256 + 256],
                               mc == 0, False)
                        mm(p1[:, fo * 256:fo * 256 + 256],
                           mb1r[0:2, fc * 128:fc * 128 + 128],
                           onesr[0:2, 0:256], False, True)
                    nc.scalar.activation(y1g[:], p1[:], AF.Gelu)
                    for fo in range(2):
                        fc = fp * 2 + fo
                        for lc in range(2):
                            mm(py[:, lc * 256:lc * 256 + 256],
                               y1g[:, fo * 256 + lc * 128:fo * 256 + lc * 128 + 128],
                               mw2T[:, fc * 256:(fc + 1) * 256],
                               fp == 0 and fo == 0, False)
                for lc in range(2):
                    mm(py[:, lc * 256:lc * 256 + 256], ones2[0:2, 0:128],
                       mrowb[0:2, 256:512], False, True)
                    nc.vector.tensor_add(
                        E_bf[:, h * 512 + lc * 256:h * 512 + lc * 256 + 256],
                        py[:, lc * 256:lc * 256 + 256],
                        hmt[h][:, lc * 256:(lc + 1) * 256])

            # ---- S16: X = exp(wnpre) (all heads; E slots hold wnpre) ----
            X_bf = qkp.tile([128, H * 512], bf, tag="qk", name="xb")
            for h in range(H):
                nc.scalar.activation(
                    X_bf[:, h * 512:(h + 1) * 512],
                    E_bf[:, h * 512:(h + 1) * 512], AF.Exp)

            # ---- S17: Z rows (for bias fold) and 1/Z columns via PE ----
            zrowb_t = zwp.tile([2, H * 256], bf, tag="zrowb", name="zrowb")
            nc.gpsimd.memset(zrowb_t[:], 0.0)
            for hp2 in range(4):
                pz = ps.tile([128, 512], f32, tag="ps", name="ps")
                for hh2 in range(2):
                    h = hp2 * 2 + hh2
                    for lc in range(2):
                        mm(pz[0:1, hh2 * 256:hh2 * 256 + 256],
                           onescol[0:128, 0:1],
                           X_bf[:, h * 512 + lc * 256:h * 512 + lc * 256 + 256],
                           lc == 0, lc == 1)
                nc.vector.tensor_copy(
                    zrowb_t[0:1, hp2 * 512:hp2 * 512 + 512], pz[0:1, 0:512])
            pzc = ps.tile([128, 512], f32, tag="ps", name="ps")
            for h in range(H):
                for mc2 in range(2):
                    cc = h * 2 + mc2
                    for lc in range(2):
                        mm(pzc[:, cc:cc + 1],
                           X_bf[:, h * 512 + lc * 256 + mc2 * 128:
                                h * 512 + lc * 256 + mc2 * 128 + 128],
                           onescol[0:128, 0:1], lc == 0, lc == 1)
            izc_t = zwp.tile([128, 16], f32, tag="izc", name="izc")
            nc.vector.reciprocal(izc_t[:], pzc[:, 0:16])

            # ---- S18: o_new (TM) + residual -> hres ----
            hres = hpool.tile([128, 2 * C], f32, tag="h", name="h")
            for h in range(H):
                p = ps.tile([128, 512], f32, tag="ps", name="ps")
                for mc in range(2):
                    for lc in range(2):
                        mm(p[:, mc * 256:mc * 256 + 96],
                           X_bf[:, h * 512 + lc * 256 + mc * 128:
                                h * 512 + lc * 256 + mc * 128 + 128],
                           y_bf[:, lc * C + 96 * h:lc * C + 96 * h + 96],
                           lc == 0, False)
                    mm(p[:, mc * 256:mc * 256 + 96],
                       zrowb_t[0:2, h * 256 + mc * 128:h * 256 + mc * 128 + 128],
                       rowb_t[0:2, 96 * h:96 * h + 96], False, True)
                    nc.vector.scalar_tensor_tensor(
                        hres[:, mc * C + 96 * h:mc * C + 96 * h + 96],
                        p[:, mc * 256:mc * 256 + 96],
                        izc_t[:, h * 2 + mc:h * 2 + mc + 1],
                        h_t[:, mc * C + 96 * h:mc * C + 96 * h + 96],
                        ALU.mult, ALU.add)

            # ---- outer LN2 + MLP ----
            hln2 = ln_transpose_outer(hres, 12, 18, 1e-5, "hln", hlnp)
            h_next = hpool.tile([128, 2 * C], f32, tag="h", name="h")
            py2 = []
            for lc in range(2):
                for n0, nw in ((0, 512), (512, 256)):
                    py2.append((lc, n0, nw,
                                psy.tile([128, 512], f32, tag="psy", name="psy"), 0))
            for piece in range(4):
                w1p = ww1.tile([128, 6 * C], bf, tag="ww1", name="ww1")
                nc.sync.dma_start(
                    w1p[:], dram["w1T"][li, :, piece * 4608:(piece + 1) * 4608])
                w2p = ww2.tile([128, 6 * C], bf, tag="ww2", name="ww2")
                nc.sync.dma_start(
                    w2p[:], dram["w2T"][li, :, piece * 4608:(piece + 1) * 4608])
                for fcl in range(6):
                    fc = piece * 6 + fcl
                    p1 = ps.tile([128, 512], f32, tag="ps", name="ps")
                    for cb in range(6):
                        mm(p1[:, 0:256],
                           w1p[:, fcl * C + cb * 128:fcl * C + cb * 128 + 128],
                           hln2[:, cb * 256:(cb + 1) * 256], cb == 0, cb == 5)
                    y1g = y1p.tile([128, 256], bf, tag="y1", name="y1")
                    nc.scalar.activation(y1g[:], p1[:, 0:256], AF.Gelu,
                                         bias=b1f_t[:, fc:fc + 1])
                    for lc, n0, nw, pt, po in py2:
                        mm(pt[:, po:po + nw], y1g[:, lc * 128:lc * 128 + 128],
                           w2p[:, fcl * C + n0:fcl * C + n0 + nw],
                           fc == 0, False)
            for lc, n0, nw, pt, po in py2:
                mm(pt[:, po:po + nw], ones2[0:2, 0:128],
                   rowb_t[0:2, C + n0:C + n0 + nw], False, True)
                nc.vector.tensor_add(
                    h_next[:, lc * C + n0:lc * C + n0 + nw], pt[:, po:po + nw],
                    hres[:, lc * C + n0:lc * C + n0 + nw])
            h_t = h_next

        if dbg_d is not None:
            nc.sync.dma_start(dbg_d[:], h_t[:])
        if not DBG_EPI:
            logits = scr.tile([10, 1], f32, tag="logits", name="logits")
            nc.gpsimd.memset(logits[:], 0.0)
            nc.sync.dma_start(out_d[:], logits[:])
            return
        # ---------------- epilogue ----------------
        # pooled^T (FM fold [128, 6]) = mean over tokens
        pooled = scr.tile([128, 8], f32, tag="pooled", name="pooled")
        hbf = rbp.tile([128, 2 * C], bf, tag="lnsub", name="hfin")
        nc.vector.tensor_copy(hbf[:], h_t[:])
        for cb in range(6):
            p = psy.tile([128, 512], f32, tag="psy", name="psy")
            for lc in range(2):
                mm(p[:, 0:1], hbf[:, lc * C + cb * 128:lc * C + cb * 128 + 128],
                   onescol[0:128, 0:1], lc == 0, lc == 1)
            nc.vector.tensor_scalar_mul(pooled[:, cb:cb + 1], p[:, 0:1],
                                        1.0 / L)
        # LN over all 768 (partition+fold): stats via f32 matmuls
        sq = scr.tile([128, 8], bf, tag="pooledsq", name="pooledsq")
        sqa = st()
        nc.vector.scalar_tensor_tensor(
            sq[:, 0:6], pooled[:, 0:6], 1.0, pooled[:, 0:6], ALU.mult,
            ALU.mult, accum_out=sqa[:])
        sqab = stp.tile([128, 1], bf, tag="stb", name="stb")
        nc.vector.tensor_copy(sqab[:], sqa[:])
        sma = st()
        nc.vector.reduce_sum(sma[:], pooled[:, 0:6], axis=X_AXIS)
        smab = stp.tile([128, 1], bf, tag="stb", name="stb")
        nc.vector.tensor_copy(smab[:], sma[:])
        pst = psy.tile([128, 512], f32, tag="psy", name="psy")
        mm(pst[0:1, 0:1], smab[:], onescol[0:128, 0:1], True, True)
        mm(pst[0:1, 1:2], sqab[:], onescol[0:128, 0:1], True, True)
        stat2 = zp.tile([1, 2], f32, tag="st2", name="st2")
        nc.vector.tensor_copy(stat2[:], pst[0:1, 0:2])
        mean = zp.tile([1, 2], f32, tag="mv2", name="mv2")
        nc.vector.tensor_scalar_mul(mean[:, 0:1], stat2[:, 0:1], 1.0 / C)
        m2 = zp.tile([1, 1], f32, tag="m2", name="m2")
        nc.vector.tensor_tensor(m2[:], mean[:, 0:1], mean[:, 0:1], ALU.mult)
        var = zp.tile([1, 1], f32, tag="var", name="var")
        nc.vector.scalar_tensor_tensor(var[:], stat2[:, 1:2], 1.0 / C, m2[:],
                                       ALU.mult, ALU.subtract)
        sd = zp.tile([1, 1], f32, tag="sd", name="sd")
        nc.scalar.activation(sd[:], var[:], AF.Sqrt, bias=eps5[0:1, :])
        rr = zp.tile([1, 1], f32, tag="rr", name="rr")
        nc.vector.reciprocal(rr[:], sd[:])
        mrb = zp.tile([2, 2], bf, tag="mrb", name="mrb")
        nc.gpsimd.memset(mrb[:], 0.0)
        nc.vector.tensor_copy(mrb[0:1, 0:1], mean[:, 0:1])
        nc.vector.tensor_copy(mrb[0:1, 1:2], rr[:])
        # broadcast mean, rstd to [128, 1] via K=1 f32... use bf16 matmul
        pbc = psy.tile([128, 512], f32, tag="psy", name="psy")
        mm(pbc[0:128, 0:2], ones2[0:2, 0:128], mrb[0:2, 0:2], True, True)
        mbc = st()
        nc.vector.tensor_copy(mbc[:], pbc[0:128, 0:1])
        rbc = st()
        nc.vector.tensor_copy(rbc[:], pbc[0:128, 1:2])
        pn = scr.tile([128, 8], f32, tag="pn", name="pn")
        nc.vector.tensor_scalar(pn[:, 0:6], pooled[:, 0:6], mbc[:], rbc[:],
                                ALU.subtract, ALU.mult)
        nc.vector.tensor_tensor(pn[:, 0:6], pn[:, 0:6], normgb[:, 0:6],
                                ALU.mult)
        nc.vector.tensor_add(pn[:, 0:6], pn[:, 0:6], normgb[:, 6:12])
        # head (f32 matmuls)
        ph = psy.tile([128, 512], f32, tag="psy", name="psy")
        for cb in range(6):
            mm(ph[0:10, 0:1], headwT[:, cb * 10:(cb + 1) * 10],
               pn[:, cb:cb + 1], cb == 0, cb == 5)
        logits = scr.tile([10, 1], f32, tag="logits", name="logits")
        nc.vector.tensor_add(logits[:], ph[0:10, 0:1], headb[0:10, 0:1])
        nc.sync.dma_start(out_d[:], logits[:])


_NC_CACHE = {}
TRACE = False
LAST = {}


def _get_nc(in_map):
    key = "k"
    if key not in _NC_CACHE:
        _NC_CACHE[key] = build(in_map)
    return _NC_CACHE[key]


def kernel(**inputs):
    per_core = marshal(inputs)
    nc = _get_nc(per_core[0])
    res = run_bass_kernel_spmd(nc, per_core, core_ids=list(range(N_CORES)),
                               trace=TRACE)
    LAST["exec_time_ns"] = res.exec_time_ns
    out = np.stack([res.results[b]["out"][:, 0] for b in range(N_CORES)])
    return out.astype(np.float32)



# revision 18
# speedup vs baseline: 1.2288x; 1.2288x over previous
"""Trainium2 Bass kernel for nn_CIFARViT: 8-layer ViT with a per-head
mini-transformer over attention maps. Data-parallel: one batch element
per NeuronCore (8 cores), full inputs in / full outputs out.

Per-core layout conventions:
  - residual stream h: token-major (TM) f32, fold [128, 2*768]
    (h[l, c] -> tile[l % 128, (l // 128)*768 + c])
  - matmul operands bf16; feature-major (FM) tensors fold the feature
    dim onto partitions in 128-row chunks.
  - weights: host-pretransposed W^T bf16 images, streamed per layer.
  - LayerNorm: free-dim stats in TM; the (x-mu)*rstd and the TM->FM
    transpose are fused into one PE matmul with rhs = diag(rstd);
    gamma/beta applied on the PSUM->SBUF copy (per-partition scalars).
  - softmax(last axis): free-dim stats. softmax(axis=-2) (wn) is kept
    unnormalized (X = exp(pre)); column sums Z via ones-matmul; 1/Z и
    Z*bias folded into the consuming einsum (per-partition scale after
    transposed contraction).
"""
import sys

sys.path.insert(0, "/opt/trn_rl_repo")

import numpy as np
import ml_dtypes

import concourse.bass as bass
import concourse.mybir as mybir
import concourse.tile as tile
from concourse import bacc
from concourse.bass_utils import run_bass_kernel_spmd

BF = ml_dtypes.bfloat16
F32 = np.float32
AF = mybir.ActivationFunctionType
ALU = mybir.AluOpType
bf = mybir.dt.bfloat16
f32 = mybir.dt.float32

H = 8
HD = 96
C = 768
L = 256
D_LAYERS = 8
F = 3072

N_CORES = 8
X_AXIS = mybir.AxisListType.X
import os
DBG_LAYERS = int(os.environ.get("KLAYERS", "8"))
DBG_TAP = os.environ.get("KTAP", "") == "1"
DBG_EPI = os.environ.get("KEPI", "1") == "1"



def _fold(wt):
    """[R, Cc] with R = 128*T -> [128, T*Cc] partition fold."""
    R, Cc = wt.shape
    T = R // 128
    return np.ascontiguousarray(
        wt.reshape(T, 128, Cc).transpose(1, 0, 2).reshape(128, T * Cc)
    )


def _foldv(v):
    T = v.shape[0] // 128
    return np.ascontiguousarray(v.reshape(T, 128).T)


def marshal(inputs):
    inp = {k: np.asarray(v) for k, v in inputs.items()}
    sh = {}
    qk_l, v_l, proj_l, w1_l, w2_l, lnp_l, b1_l, rowb_l = ([] for _ in range(8))
    for i in range(D_LAYERS):
        qkvT = inp["qkv_w"][i].T.astype(BF)  # [768, 2304]
        qkf = _fold(qkvT)  # [128, 6*2304]
        img = np.zeros((128, 4 * 2304), dtype=BF)
        for hp in range(4):
            for cb in range(6):
                for hh in range(2):
                    h = hp * 2 + hh
                    base = hp * 2304 + cb * 384 + hh * 192
                    img[:, base:base + 96] = \
                        qkf[:, cb * 2304 + 96 * h: cb * 2304 + 96 * h + 96]
                    img[:, base + 96:base + 192] = \
                        qkf[:, cb * 2304 + 768 + 96 * h: cb * 2304 + 768 + 96 * h + 96]
        qk_l.append(img)
        v_l.append(_fold(np.ascontiguousarray(qkvT[:, 1536:2304])))
        proj_l.append(_fold(inp["proj_w"][i].T.astype(BF)))
        w1T = inp["mlp_w1"][i].T.astype(BF)  # [768, 3072]
        w1_l.append(np.ascontiguousarray(
            w1T.reshape(6, 128, 24, 128).transpose(1, 2, 0, 3).reshape(128, 24 * 768)))
        w2_l.append(_fold(inp["mlp_w2"][i].T.astype(BF)))  # [128, 24*768]
        lnp_l.append(np.concatenate(
            [_foldv(inp[k][i].astype(F32))
             for k in ("ln1_g", "ln1_b", "ln2_g", "ln2_b")], axis=1))
        b1_l.append(_foldv(inp["mlp_b1"][i].astype(F32)))
        rb = np.zeros((2, 2 * C), dtype=BF)
        rb[0, :C] = inp["proj_b"][i].astype(BF)
        rb[0, C:] = inp["mlp_b2"][i].astype(BF)
        rowb_l.append(rb)
    sh["qk_img"] = np.stack(qk_l)
    sh["v_img"] = np.stack(v_l)
    sh["projT"] = np.stack(proj_l)
    sh["w1T"] = np.stack(w1_l)
    sh["w2T"] = np.stack(w2_l)
    sh["lnp"] = np.stack(lnp_l)
    sh["b1f"] = np.stack(b1_l)
    sh["rowb"] = np.stack(rowb_l)

    sh["mqkvT"] = _fold(inp["m_qkv_w"].T.astype(BF))   # [128, 2*768]
    sh["mprojT"] = _fold(inp["m_proj_w"].T.astype(BF))  # [128, 2*256]
    sh["mw1T"] = _fold(inp["m_mlp_w1"].T.astype(BF))   # [128, 2*1024]
    sh["mw2T"] = _fold(inp["m_mlp_w2"].T.astype(BF))   # [128, 8*256]
    sh["mlnp"] = np.concatenate(
        [_foldv(inp[k].astype(F32))
         for k in ("m_ln1_g", "m_ln1_b", "m_ln2_g", "m_ln2_b")], axis=1)
    sh["mb1f"] = _foldv(inp["m_mlp_b1"].astype(F32))   # [128, 8]
    mb1r_ = np.zeros((2, 1024), dtype=BF)
    mb1r_[0, :] = inp["m_mlp_b1"].astype(BF)
    sh["mb1r"] = mb1r_
    onesr_ = np.zeros((2, 256), dtype=BF)
    onesr_[0, :] = 1.0
    sh["onesr"] = onesr_
    mrb_ = np.zeros((2, 512), dtype=BF)
    mrb_[0, :256] = inp["m_proj_b"].astype(BF)
    mrb_[0, 256:] = inp["m_mlp_b2"].astype(BF)
    sh["mrowb"] = mrb_

    sh["pwT"] = np.ascontiguousarray(inp["patch_w"].reshape(C, 12).T.astype(BF))
    pos = inp["pos_emb"][0].astype(F32) + inp["patch_b"][None, :].astype(F32)
    sh["pos"] = _fold(pos)  # [128, 2*768]
    sh["normgb"] = np.concatenate(
        [_foldv(inp["norm_g"].astype(F32)), _foldv(inp["norm_b"].astype(F32))],
        axis=1)  # [128, 12]
    sh["headwT"] = _fold(inp["head_w"].T.astype(F32))  # [128, 6*10]
    sh["headb"] = inp["head_b"].astype(F32).reshape(10, 1)
    sh["ident"] = np.eye(128, dtype=BF)
    sh["onescol"] = np.ones((128, 1), dtype=BF)
    sh["ones2"] = np.ones((2, 128), dtype=BF)
    e0_ = np.zeros((2, 1), dtype=BF); e0_[0, 0] = 1.0
    sh["e0"] = e0_
    sh["onescolf"] = np.ones((128, 1), dtype=F32)

    x = inp["x"].astype(F32)
    per_core = []
    for b in range(N_CORES):
        pt = (x[b].reshape(3, 16, 2, 16, 2).transpose(0, 2, 4, 1, 3)
              .reshape(12, 256).astype(BF))
        m = dict(sh)
        m["patchesT"] = np.ascontiguousarray(pt)
        per_core.append(m)
    return per_core


DT_MAP = {np.dtype(BF): bf, np.dtype(np.float32): f32}


def build(in_map):
    nc = bacc.Bacc("TRN2", target_bir_lowering=False, debug=False,
                   num_devices=N_CORES)
    dram = {k: nc.dram_tensor(k, v.shape, DT_MAP[v.dtype], kind="ExternalInput")
            for k, v in in_map.items()}
    out_d = nc.dram_tensor("out", (10, 1), f32, kind="ExternalOutput")
    dbg_d = nc.dram_tensor("dbg", (128, 2 * C), f32,
                           kind="ExternalOutput") if DBG_TAP else None
    with tile.TileContext(nc) as tc:
        _body(nc, tc, dram, out_d, dbg_d)
    nc.compile()
    return nc


def _body(nc, tc, dram, out_d, dbg_d=None):
    import contextlib
    ctx = contextlib.ExitStack()
    with ctx:
        P = lambda name, bufs=1, space="SBUF": ctx.enter_context(
            tc.tile_pool(name=name, bufs=bufs, space=space))
        cpool = P("const")

        def cload(name):
            arr = dram[name]
            t = cpool.tile(list(arr.shape), arr.dtype, tag=name, name=name)
            nc.sync.dma_start(t[:], arr[:])
            return t

        # prologue-critical loads first so patch embed starts immediately
        pwT = cload("pwT")
        patchesT = cload("patchesT")
        mqkvT = cload("mqkvT")
        mprojT = cload("mprojT")
        mw1T = cload("mw1T")
        mw2T = cload("mw2T")
        mlnp = cload("mlnp")
        mb1f = cload("mb1f")
        mrowb = cload("mrowb")
        ident = cload("ident")
        onescol = cload("onescol")
        ones2 = cload("ones2")
        normgb = cload("normgb")
        headwT = cload("headwT")
        headb = cload("headb")

        NDIAG = 3
        dtiles = []
        for j in range(NDIAG):
            t = cpool.tile([128, 256], bf, tag=f"diag{j}", name=f"diag{j}")
            nc.gpsimd.memset(t[:], 0.0)
            dtiles.append(t)
        dctr = [0]

        eps6 = cpool.tile([128, 1], f32, tag="eps6", name="eps6")
        nc.gpsimd.memset(eps6[:], 1e-6)
        eps5 = cpool.tile([128, 1], f32, tag="eps5", name="eps5")
        nc.gpsimd.memset(eps5[:], 1e-5)
        u32 = mybir.dt.uint32
        magic_t = cpool.tile([128, 32], u32, tag="magic", name="magic")
        nc.gpsimd.memset(magic_t[:], 0x5f3759df)

        RSQ_ACT = os.environ.get("KRSQ", "") == "act"
        RSQ_IT = int(os.environ.get("KRSQIT", "1"))

        def rsqrt_cols(src_ap, w, eps):
            """[128, w] f32 -> 1/sqrt(src+eps); int-seed + 2 Newton (DVE only)."""
            if RSQ_ACT:
                sdt = rsp.tile([128, 32], f32, tag="rsq_sd", name="rsq_sd")
                if eps:
                    epst = eps6 if eps == 1e-6 else eps5
                    nc.scalar.activation(sdt[:, 0:w], src_ap, AF.Sqrt,
                                         bias=epst[:])
                else:
                    nc.scalar.activation(sdt[:, 0:w], src_ap, AF.Sqrt)
                outt = rsp.tile([128, 32], f32, tag="rsq_oo", name="rsq_oo")
                nc.vector.reciprocal(outt[:, 0:w], sdt[:, 0:w])
                return outt[:, 0:w]
            if eps:
                v = rsp.tile([128, 32], f32, tag="rsq_v", name="rsq_v")
                nc.vector.tensor_scalar_add(v[:, 0:w], src_ap, eps)
                vap = v[:, 0:w]
            else:
                vap = src_ap
            shv = rsp.tile([128, 32], u32, tag="rsq_s", name="rsq_s")
            nc.vector.tensor_scalar(shv[:, 0:w], vap.bitcast(u32), 1, None,
                                    ALU.logical_shift_right)
            y0 = rsp.tile([128, 32], u32, tag="rsq_y", name="rsq_y")
            nc.vector.tensor_tensor(y0[:, 0:w], magic_t[:, 0:w], shv[:, 0:w],
                                    ALU.subtract)
            cur = y0[:, 0:w].bitcast(f32)
            for it in range(RSQ_IT):
                t1 = rsp.tile([128, 32], f32, tag=f"rsq_t{it}", name="rsq_t")
                nc.vector.tensor_tensor(t1[:, 0:w], cur, cur, ALU.mult)
                nc.vector.tensor_tensor(t1[:, 0:w], t1[:, 0:w], vap, ALU.mult)
                nc.vector.tensor_scalar(t1[:, 0:w], t1[:, 0:w], -0.5, 1.5,
                                        ALU.mult, ALU.add)
                nxt = rsp.tile([128, 32], f32, tag=f"rsq_o{it}", name="rsq_o")
                nc.vector.tensor_tensor(nxt[:, 0:w], cur, t1[:, 0:w], ALU.mult)
                cur = nxt[:, 0:w]
            return cur

        def build_diag(s0, s1):
            d = dtiles[dctr[0] % NDIAG]
            dctr[0] += 1
            nc.vector.tensor_scalar_mul(d[:, 0:128], ident[:], s0)
            nc.vector.tensor_scalar_mul(d[:, 128:256], ident[:], s1)
            return d

        hpool = P("h", bufs=3)
        ps = P("ps", bufs=4, space="PSUM")
        psy = P("psy", bufs=4, space="PSUM")
        stp = P("st", bufs=16)
        wqk = P("wqk", bufs=2)
        wv = P("wv", bufs=1)
        wproj = P("wproj", bufs=1)
        ww1 = P("ww1", bufs=3)
        ww2 = P("ww2", bufs=3)
        hlnp = P("hln", bufs=1)
        qkp = P("qkt", bufs=1)
        vtp = P("vt", bufs=1)
        ebp = P("eb", bufs=1)
        atp = P("at", bufs=1)
        ybp = P("yb", bufs=1)
        scr = P("scr", bufs=2)
        wnp = P("wn", bufs=8)
        mqp = P("mq", bufs=8)
        mvp = P("mvp", bufs=8)
        emp = P("emp", bufs=8)
        omp = P("omp", bufs=8)
        hmp = P("hm", bufs=8)
        rsp = P("rs", bufs=2)
        rbp = P("rb", bufs=1)
        rowbp = P("rowb", bufs=2)
        y1p = P("y1", bufs=2)
        zp = P("zp", bufs=2)
        zwp = P("zw", bufs=1)

        def st():
            return stp.tile([128, 1], f32, tag="st", name="st")

        def mm(out, lhsT, rhs, start, stop):
            nc.tensor.matmul(out, lhsT, rhs, start=start, stop=stop)

        # ---------------- prologue: patch embed ----------------
        pos_t = hpool.tile([128, 2 * C], f32, tag="h", name="pos")
        h_t = hpool.tile([128, 2 * C], f32, tag="h", name="h")
        nc.sync.dma_start(pos_t[:], dram["pos"][:])
        for lc in range(2):
            for n0, nw in ((0, 512), (512, 256)):
                p = ps.tile([128, 512], f32, tag="ps", name="ps")
                mm(p[:, 0:nw], patchesT[0:12, lc * 128:lc * 128 + 128],
                   pwT[0:12, n0:n0 + nw], True, True)
                nc.vector.tensor_add(
                    h_t[:, lc * C + n0:lc * C + n0 + nw], p[:, 0:nw],
                    pos_t[:, lc * C + n0:lc * C + n0 + nw])

        # ---------------- layers ----------------
        for li in range(DBG_LAYERS):
            lnp_t = scr.tile([128, 24], f32, tag="lnp", name="lnp")
            nc.sync.dma_start(lnp_t[:], dram["lnp"][li])
            b1f_t = scr.tile([128, 24], f32, tag="b1f", name="b1f")
            nc.sync.dma_start(b1f_t[:], dram["b1f"][li])
            rowb_t = rowbp.tile([2, 2 * C], bf, tag="rowb", name="rowb")
            nc.sync.dma_start(rowb_t[:], dram["rowb"][li])
            vt_w = wv.tile([128, 6 * C], bf, tag="wv", name="wv")
            nc.sync.dma_start(vt_w[:], dram["v_img"][li])
            projT = wproj.tile([128, 6 * C], bf, tag="wproj", name="wproj")
            nc.sync.dma_start(projT[:], dram["projT"][li])

            def ln_transpose_outer(src, g_col, b_col, epsv, tag, pool):
                """TM f32 [128, 2*768] -> LN'd FM bf16 [128, 6*256]."""
                dst = pool.tile([128, 6 * 256], bf, tag=tag)
                subs = rbp.tile([128, 2 * C], bf, tag="lnsub", name="lnsub")
                mv4 = zwp.tile([128, 4], f32, tag="mv4", name="mv4")
                for lc in range(2):
                    bns = stp.tile([128, 12], f32, tag="bns12", name="bns12")
                    nc.vector.bn_stats(
                        bns[:, 0:6], src[:, lc * C:lc * C + 384])
                    nc.vector.bn_stats(
                        bns[:, 6:12], src[:, lc * C + 384:(lc + 1) * C])
                    nc.vector.bn_aggr(mv4[:, lc * 2:lc * 2 + 2], bns[:])
                rs4 = rsqrt_cols(mv4[:], 4, epsv)
                for lc in range(2):
                    nc.vector.tensor_scalar_sub(
                        subs[:, lc * C:(lc + 1) * C],
                        src[:, lc * C:(lc + 1) * C],
                        mv4[:, lc * 2:lc * 2 + 1])
                d = build_diag(rs4[:, 1:2], rs4[:, 3:4])
                for cb in range(6):
                    p = ps.tile([128, 512], f32, tag="ps", name="ps")
                    for lc in range(2):
                        mm(p[:, lc * 128:lc * 128 + 128],
                           subs[:, lc * C + cb * 128:lc * C + cb * 128 + 128],
                           d[:, lc * 128:lc * 128 + 128], True, True)
                    nc.vector.tensor_scalar(
                        dst[:, cb * 256:(cb + 1) * 256], p[:, 0:256],
                        lnp_t[:, g_col + cb:g_col + cb + 1],
                        lnp_t[:, b_col + cb:b_col + cb + 1],
                        ALU.mult, ALU.add)
                return dst

            hln = ln_transpose_outer(h_t, 0, 6, 1e-6, "hln", hlnp)

            # ---- qkv: q^T|k^T per head [96, 512]; v token-major ----
            qk_bf = qkp.tile([128, H * 512], bf, tag="qk", name="qk")
            for hp in range(4):
                qkw = wqk.tile([128, 2304], bf, tag="wqk", name="wqk")
                nc.sync.dma_start(
                    qkw[:], dram["qk_img"][li, :, hp * 2304:(hp + 1) * 2304])
                for hh in range(2):
                    h = hp * 2 + hh
                    p = ps.tile([128, 512], f32, tag="ps", name="ps")
                    for half in range(2):
                        for cb in range(6):
                            lh = qkw[:, cb * 384 + hh * 192 + half * 96:
                                     cb * 384 + hh * 192 + half * 96 + 96]
                            mm(p[0:96, half * 256:half * 256 + 256], lh,
                               hln[:, cb * 256:(cb + 1) * 256],
                               cb == 0, cb == 5)
                    nc.scalar.copy(
                        qk_bf[0:96, h * 512:(h + 1) * 512], p[0:96, :])
            vt_bf = vtp.tile([128, 2 * C], bf, tag="vt", name="vt")
            for lc in range(2):
                for n0, nw in ((0, 512), (512, 256)):
                    p = ps.tile([128, 512], f32, tag="ps", name="ps")
                    for cb in range(6):
                        mm(p[:, 0:nw],
                           hln[:, cb * 256 + lc * 128:cb * 256 + lc * 128 + 128],
                           vt_w[:, cb * C + n0:cb * C + n0 + nw],
                           cb == 0, cb == 5)
                    nc.vector.tensor_copy(
                        vt_bf[:, lc * C + n0:lc * C + n0 + nw], p[:, 0:nw])

            # ---- S3: scores -> E = exp(s); Z, 1/Z (all heads) ----
            E_bf = ebp.tile([128, H * 512], bf, tag="eb", name="eb")
            aT_bf = atp.tile([128, H * 512], bf, tag="at", name="at")
            zt = zwp.tile([128, 16], f32, tag="zt", name="zt")
            izt = zwp.tile([128, 16], f32, tag="izt", name="izt")
            for h in range(H):
                p = ps.tile([128, 512], f32, tag="ps", name="ps")
                for lc in range(2):
                    mm(p[:, lc * 256:lc * 256 + 256],
                       qk_bf[0:96, h * 512 + lc * 128:h * 512 + lc * 128 + 128],
                       qk_bf[0:96, h * 512 + 256:h * 512 + 512], True, True)
                    nc.scalar.activation(
                        E_bf[:, h * 512 + lc * 256:h * 512 + lc * 256 + 256],
                        p[:, lc * 256:lc * 256 + 256], AF.Exp, scale=HD ** -0.5,
                        accum_out=zt[:, h * 2 + lc:h * 2 + lc + 1])
                nc.vector.reciprocal(izt[:, h * 2:h * 2 + 2],
                                     zt[:, h * 2:h * 2 + 2])
            # ---- S4: aT = (E/Z)^T (all heads) ----
            for h in range(H):
                d = build_diag(izt[:, h * 2:h * 2 + 1],
                               izt[:, h * 2 + 1:h * 2 + 2])
                pa = ps.tile([128, 512], f32, tag="ps", name="ps")
                for mc in range(2):
                    for lc in range(2):
                        mm(pa[:, mc * 256 + lc * 128:mc * 256 + lc * 128 + 128],
                           E_bf[:, h * 512 + lc * 256 + mc * 128:
                                h * 512 + lc * 256 + mc * 128 + 128],
                           d[:, lc * 128:lc * 128 + 128], True, True)
                nc.scalar.copy(
                    aT_bf[:, h * 512:(h + 1) * 512], pa[:])
            # ---- S5: attnV -> oT ----
            oT_bf = hlnp.tile([128, 6 * 256], bf, tag="ot", name="ot")
            for h in range(H):
                po = ps.tile([128, 512], f32, tag="ps", name="ps")
                for mc in range(2):
                    mm(po[0:96, 0:256],
                       vt_bf[:, mc * C + 96 * h:mc * C + 96 * h + 96],
                       aT_bf[:, h * 512 + mc * 256:h * 512 + mc * 256 + 256],
                       mc == 0, mc == 1)
                for k in range(3):
                    rr = 96 * h + 32 * k
                    cb, r0 = divmod(rr, 128)
                    nc.vector.tensor_copy(
                        oT_bf[r0:r0 + 32, cb * 256:(cb + 1) * 256],
                        po[32 * k:32 * k + 32, 0:256])

            # ---- S6: proj -> y (TM bf16); proj_b folded in here (softmax
            # over l sums to 1, so o-mix passes the bias through exactly) ----
            y_bf = ybp.tile([128, 2 * C], bf, tag="yb", name="yb")
            for lc in range(2):
                for n0, nw in ((0, 512), (512, 256)):
                    p = ps.tile([128, 512], f32, tag="ps", name="ps")
                    for cb in range(6):
                        mm(p[:, 0:nw],
                           oT_bf[:, cb * 256 + lc * 128:cb * 256 + lc * 128 + 128],
                           projT[:, cb * C + n0:cb * C + n0 + nw],
                           cb == 0, False)
                    mm(p[:, 0:nw], ones2[0:2, 0:128],
                       rowb_t[0:2, n0:n0 + nw], False, True)
                    nc.scalar.copy(
                        y_bf[:, lc * C + n0:lc * C + n0 + nw], p[:, 0:nw])

            # ---- S7: mini LN1 (batched stats over all heads) ----
            # LN of a = E/Z without normalizing: mean_a = 1/L exactly;
            # var_a = sumsq(E)*iz^2/L - 1/L^2. Centering const zc = Z/L.
            sqt = zwp.tile([128, 16], f32, tag="sqt", name="sqt")
            for h in range(H):
                for lc in range(2):
                    esl = E_bf[:, h * 512 + lc * 256:h * 512 + lc * 256 + 256]
                    sqs = scr.tile([128, 256], bf, tag="msq", name="msq")
                    nc.vector.scalar_tensor_tensor(
                        sqs[:], esl, 1.0, esl, ALU.mult, ALU.mult,
                        accum_out=sqt[:, h * 2 + lc:h * 2 + lc + 1])
            iz2t = zwp.tile([128, 16], f32, tag="iz2t", name="iz2t")
            nc.vector.tensor_tensor(iz2t[:], izt[:], izt[:], ALU.mult)
            nc.vector.tensor_tensor(iz2t[:], sqt[:], iz2t[:], ALU.mult)
            vart = zwp.tile([128, 16], f32, tag="vart", name="vart")
            nc.vector.tensor_scalar(
                vart[:], iz2t[:], 1.0 / L, 1.0 / (L * L) - 1e-6,
                ALU.mult, ALU.subtract)
            ra1 = rsqrt_cols(vart[:], 16, 0.0)
            rzt = zwp.tile([128, 16], f32, tag="rzt", name="rzt")
            nc.vector.tensor_tensor(rzt[:], ra1, izt[:], ALU.mult)
            zct = zwp.tile([128, 16], f32, tag="zct", name="zct")
            nc.vector.tensor_scalar_mul(zct[:], zt[:], 1.0 / L)
            wnl = []
            for h in range(H):
                esub = scr.tile([128, 512], bf, tag="esub", name="esub")
                for lc in range(2):
                    nc.vector.tensor_scalar_sub(
                        esub[:, lc * 256:(lc + 1) * 256],
                        E_bf[:, h * 512 + lc * 256:h * 512 + lc * 256 + 256],
                        zct[:, h * 2 + lc:h * 2 + lc + 1])
                d = build_diag(rzt[:, h * 2:h * 2 + 1],
                               rzt[:, h * 2 + 1:h * 2 + 2])
                pw_ = ps.tile([128, 512], f32, tag="ps", name="ps")
                for mc in range(2):
                    for lc in range(2):
                        mm(pw_[:, mc * 256 + lc * 128:mc * 256 + lc * 128 + 128],
                           esub[:, lc * 256 + mc * 128:lc * 256 + mc * 128 + 128],
                           d[:, lc * 128:lc * 128 + 128], True, True)
                wnl_h = wnp.tile([128, 512], bf, tag="wn", name="wn")
                for mc in range(2):
                    nc.vector.tensor_scalar(
                        wnl_h[:, mc * 256:mc * 256 + 256],
                        pw_[:, mc * 256:mc * 256 + 256],
                        mlnp[:, 0 + mc:1 + mc], mlnp[:, 2 + mc:3 + mc],
                        ALU.mult, ALU.add)
                wnl.append(wnl_h)

            # ---- S8: mini qkv (q|k interleaved per dc) + v ----
            mqk = []
            mv = []
            for h in range(H):
                mqk_h = mqp.tile([128, 1024], bf, tag="mqk", name="mqk")
                for dc in range(2):
                    p = ps.tile([128, 512], f32, tag="ps", name="ps")
                    for half in range(2):
                        for mc in range(2):
                            mm(p[:, half * 256:half * 256 + 256],
                               mqkvT[:, mc * 768 + half * 256 + dc * 128:
                                     mc * 768 + half * 256 + dc * 128 + 128],
                               wnl[h][:, mc * 256:mc * 256 + 256],
                               mc == 0, mc == 1)
                    nc.vector.tensor_copy(
                        mqk_h[:, dc * 512:dc * 512 + 512], p[:])
                mv_h = mvp.tile([128, 512], bf, tag="mv", name="mv")
                p2 = ps.tile([128, 512], f32, tag="ps", name="ps")
                for lc in range(2):
                    for mc in range(2):
                        mm(p2[:, lc * 256:lc * 256 + 256],
                           wnl[h][:, mc * 256 + lc * 128:mc * 256 + lc * 128 + 128],
                           mqkvT[:, mc * 768 + 512:mc * 768 + 768],
                           mc == 0, mc == 1)
                nc.scalar.copy(mv_h[:], p2[:])
                mqk.append(mqk_h)
                mv.append(mv_h)

            # ---- S9: mini scores -> Em = exp(s); Z, 1/Z ----
            mzt = zwp.tile([128, 16], f32, tag="mzt", name="mzt")
            mizt = zwp.tile([128, 16], f32, tag="mizt", name="mizt")
            Em = []
            for h in range(H):
                Em_h = emp.tile([128, 512], bf, tag="em", name="em")
                p = ps.tile([128, 512], f32, tag="ps", name="ps")
                for lc in range(2):
                    for dc in range(2):
                        mm(p[:, lc * 256:lc * 256 + 256],
                           mqk[h][:, dc * 512 + lc * 128:dc * 512 + lc * 128 + 128],
                           mqk[h][:, dc * 512 + 256:dc * 512 + 512],
                           dc == 0, dc == 1)
                    nc.scalar.activation(
                        Em_h[:, lc * 256:lc * 256 + 256],
                        p[:, lc * 256:lc * 256 + 256], AF.Exp, scale=0.0625,
                        accum_out=mzt[:, h * 2 + lc:h * 2 + lc + 1])
                nc.vector.reciprocal(mizt[:, h * 2:h * 2 + 2],
                                     mzt[:, h * 2:h * 2 + 2])
                Em.append(Em_h)
            # b1g = m_mlp_b1 columns + 0*mzt: numerically mb1f, but depends
            # on ALL S9 exps so the scheduler can't interleave S14 Gelu with
            # S9 Exp (each interleave costs a ~2.7us ACT table switch).
            b1g = zwp.tile([128, 8], f32, tag="b1g", name="b1g")
            nc.vector.scalar_tensor_tensor(
                b1g[:], mzt[:, 0:8], 0.0, mzt[:, 8:16], ALU.mult, ALU.mult)
            nc.vector.tensor_add(b1g[:], b1g[:], mb1f[:, 0:8])

            # ---- S10: amT = (Em/Z)^T ----
            amT = []
            for h in range(H):
                d = build_diag(mizt[:, h * 2:h * 2 + 1],
                               mizt[:, h * 2 + 1:h * 2 + 2])
                pa = ps.tile([128, 512], f32, tag="ps", name="ps")
                for mc in range(2):
                    for lc in range(2):
                        mm(pa[:, mc * 256 + lc * 128:mc * 256 + lc * 128 + 128],
                           Em[h][:, lc * 256 + mc * 128:lc * 256 + mc * 128 + 128],
                           d[:, lc * 128:lc * 128 + 128], True, True)
                amT_h = wnp.tile([128, 512], bf, tag="wn", name="amt")
                nc.vector.tensor_copy(amT_h[:], pa[:])
                amT.append(amT_h)

            # ---- S11: mini attnV -> omT (FM) ----
            omT = []
            for h in range(H):
                po = ps.tile([128, 512], f32, tag="ps", name="ps")
                for dc in range(2):
                    for mc in range(2):
                        mm(po[:, dc * 256:dc * 256 + 256],
                           mv[h][:, mc * 256 + dc * 128:mc * 256 + dc * 128 + 128],
                           amT[h][:, mc * 256:mc * 256 + 256],
                           mc == 0, mc == 1)
                omT_h = omp.tile([128, 512], bf, tag="om", name="om")
                nc.vector.tensor_copy(omT_h[:], po[:])
                omT.append(omT_h)

            # ---- S12: hmini = omT.T@mprojT + aT.T@mprojT + pb (TM bf16) ----
            hmt = []
            for h in range(H):
                ph = ps.tile([128, 512], f32, tag="ps", name="ps")
                for lc in range(2):
                    for mc in range(2):
                        mm(ph[:, lc * 256:lc * 256 + 256],
                           omT[h][:, mc * 256 + lc * 128:mc * 256 + lc * 128 + 128],
                           mprojT[:, mc * 256:(mc + 1) * 256],
                           mc == 0, False)
                    for mc in range(2):
                        mm(ph[:, lc * 256:lc * 256 + 256],
                           aT_bf[:, h * 512 + mc * 256 + lc * 128:
                                 h * 512 + mc * 256 + lc * 128 + 128],
                           mprojT[:, mc * 256:(mc + 1) * 256],
                           False, False)
                    mm(ph[:, lc * 256:lc * 256 + 256], ones2[0:2, 0:128],
                       mrowb[0:2, 0:256], False, True)
                hm_h = hmp.tile([128, 512], bf, tag="hm", name="hm")
                nc.scalar.copy(hm_h[:], ph[:])
                hmt.append(hm_h)

            # ---- S13: mini LN2 (bn stats; batched rsqrt) ----
            mv2 = zwp.tile([128, 32], f32, tag="mv2", name="mv2")
            for h in range(H):
                for lc in range(2):
                    bns = stp.tile([128, 6], f32, tag="bns6", name="bns6")
                    nc.vector.bn_stats(
                        bns[:], hmt[h][:, lc * 256:(lc + 1) * 256])
                    cc = (h * 2 + lc) * 2
                    nc.vector.bn_aggr(mv2[:, cc:cc + 2], bns[:])
            rs2 = rsqrt_cols(mv2[:], 32, 1e-5)
            mhln = []
            for h in range(H):
                hsub = scr.tile([128, 512], bf, tag="esub", name="hsub")
                for lc in range(2):
                    cc = (h * 2 + lc) * 2
                    nc.vector.tensor_scalar_sub(
                        hsub[:, lc * 256:(lc + 1) * 256],
                        hmt[h][:, lc * 256:(lc + 1) * 256],
                        mv2[:, cc:cc + 1])
                d = build_diag(rs2[:, h * 4 + 1:h * 4 + 2],
                               rs2[:, h * 4 + 3:h * 4 + 4])
                ph2 = ps.tile([128, 512], f32, tag="ps", name="ps")
                for mc in range(2):
                    for lc in range(2):
                        mm(ph2[:, mc * 256 + lc * 128:mc * 256 + lc * 128 + 128],
                           hsub[:, lc * 256 + mc * 128:lc * 256 + mc * 128 + 128],
                           d[:, lc * 128:lc * 128 + 128], True, True)
                mhln_h = emp.tile([128, 512], bf, tag="em", name="mhln")
                for mc in range(2):
                    nc.vector.tensor_scalar(
                        mhln_h[:, mc * 256:mc * 256 + 256],
                        ph2[:, mc * 256:mc * 256 + 256],
                        mlnp[:, 4 + mc:5 + mc], mlnp[:, 6 + mc:7 + mc],
                        ALU.mult, ALU.add)
                mhln.append(mhln_h)

            # ---- S14/S15: mini MLP (gelu cluster) + wnpre into E slots.
            # b1 applied via the Gelu bias operand (per-partition column);
            # m_mlp_b2 dropped entirely: constant over l, cancels in the
            # axis=-2 softmax. ----
            for h in range(H):
                py = psy.tile([128, 512], f32, tag="psy", name="psy")
                for fp in range(4):
                    p1 = ps.tile([128, 512], f32, tag="ps", name="ps")
                    y1g = y1p.tile([128, 512], bf, tag="y1", name="y1")
                    for fo in range(2):
                        fc = fp * 2 + fo
                        for mc in range(2):
                            mm(p1[:, fo * 256:fo * 256 + 256],
                               mw1T[:, mc * 1024 + fc * 128:
                                    mc * 1024 + fc * 128 + 128],
                               mhln[h][:, mc * 256:mc * 256 + 256],
                               mc == 0, mc == 1)
                    for fo in range(2):
                        fc = fp * 2 + fo
                        nc.scalar.activation(
                            y1g[:, fo * 256:fo * 256 + 256],
                            p1[:, fo * 256:fo * 256 + 256], AF.Gelu,
                            bias=b1g[:, fc:fc + 1])
                    for fo in range(2):
                        fc = fp * 2 + fo
                        for lc in range(2):
                            mm(py[:, lc * 256:lc * 256 + 256],
                               y1g[:, fo * 256 + lc * 128:fo * 256 + lc * 128 + 128],
                               mw2T[:, fc * 256:(fc + 1) * 256],
                               fp == 0 and fo == 0, fp == 3 and fo == 1)
                for lc in range(2):
                    nc.vector.tensor_add(
                        E_bf[:, h * 512 + lc * 256:h * 512 + lc * 256 + 256],
                        py[:, lc * 256:lc * 256 + 256],
                        hmt[h][:, lc * 256:(lc + 1) * 256])

            # ---- S16: X = exp(wnpre), one op over all heads (single dep on
            # every S14 add -> exactly one Gelu->Exp table switch here) ----
            X_bf = qkp.tile([128, H * 512], bf, tag="qk", name="xb")
            nc.scalar.activation(X_bf[:], E_bf[:], AF.Exp)

            # ---- S17: 1/Z columns via PE (proj_b already in y, so the Z-row
            # bias fold is gone) ----
            pzc = ps.tile([128, 512], f32, tag="ps", name="ps")
            for h in range(H):
                for mc2 in range(2):
                    cc = h * 2 + mc2
                    for lc in range(2):
                        mm(pzc[:, cc:cc + 1],
                           X_bf[:, h * 512 + lc * 256 + mc2 * 128:
                                h * 512 + lc * 256 + mc2 * 128 + 128],
                           onescol[0:128, 0:1], lc == 0, lc == 1)
            izc_t = zwp.tile([128, 16], f32, tag="izc", name="izc")
            nc.vector.reciprocal(izc_t[:], pzc[:, 0:16])

            # ---- S18: o_new (TM) + residual -> hres (mc-major so LN2 stats
            # on token-chunk 0 start while chunk 1 is still computing) ----
            hres = hpool.tile([128, 2 * C], f32, tag="h", name="h")
            for mc in range(2):
                for h in range(H):
                    p = ps.tile([128, 512], f32, tag="ps", name="ps")
                    for lc in range(2):
                        mm(p[:, 0:96],
                           X_bf[:, h * 512 + lc * 256 + mc * 128:
                                h * 512 + lc * 256 + mc * 128 + 128],
                           y_bf[:, lc * C + 96 * h:lc * C + 96 * h + 96],
                           lc == 0, lc == 1)
                    nc.vector.scalar_tensor_tensor(
                        hres[:, mc * C + 96 * h:mc * C + 96 * h + 96],
                        p[:, 0:96],
                        izc_t[:, h * 2 + mc:h * 2 + mc + 1],
                        h_t[:, mc * C + 96 * h:mc * C + 96 * h + 96],
                        ALU.mult, ALU.add)

            # ---- outer LN2 + MLP ----
            hln2 = ln_transpose_outer(hres, 12, 18, 1e-5, "hln", hlnp)
            h_next = hpool.tile([128, 2 * C], f32, tag="h", name="h")
            py2 = []
            for lc in range(2):
                for n0, nw in ((0, 512), (512, 256)):
                    py2.append((lc, n0, nw,
                                psy.tile([128, 512], f32, tag="psy", name="psy"), 0))
            for piece in range(8):
                w1p = ww1.tile([128, 3 * C], bf, tag="ww1", name="ww1")
                nc.sync.dma_start(
                    w1p[:], dram["w1T"][li, :, piece * 2304:(piece + 1) * 2304])
                w2p = ww2.tile([128, 3 * C], bf, tag="ww2", name="ww2")
                nc.sync.dma_start(
                    w2p[:], dram["w2T"][li, :, piece * 2304:(piece + 1) * 2304])
                for fcl in range(3):
                    fc = piece * 3 + fcl
                    p1 = ps.tile([128, 512], f32, tag="ps", name="ps")
                    for cb in range(6):
                        mm(p1[:, 0:256],
                           w1p[:, fcl * C + cb * 128:fcl * C + cb * 128 + 128],
                           hln2[:, cb * 256:(cb + 1) * 256], cb == 0, cb == 5)
                    y1g = y1p.tile([128, 256], bf, tag="y1", name="y1")
                    nc.scalar.activation(y1g[:], p1[:, 0:256], AF.Gelu,
                                         bias=b1f_t[:, fc:fc + 1])
                    for lc, n0, nw, pt, po in py2:
                        mm(pt[:, po:po + nw], y1g[:, lc * 128:lc * 128 + 128],
                           w2p[:, fcl * C + n0:fcl * C + n0 + nw],
                           fc == 0, False)
            for lc, n0, nw, pt, po in py2:
                mm(pt[:, po:po + nw], ones2[0:2, 0:128],
                   rowb_t[0:2, C + n0:C + n0 + nw], False, True)
                nc.vector.tensor_add(
                    h_next[:, lc * C + n0:lc * C + n0 + nw], pt[:, po:po + nw],
                    hres[:, lc * C + n0:lc * C + n0 + nw])
            h_t = h_next

        if dbg_d is not None:
            nc.sync.dma_start(dbg_d[:], h_t[:])
        if not DBG_EPI:
            logits = scr.tile([10, 1], f32, tag="logits", name="logits")
            nc.gpsimd.memset(logits[:], 0.0)
            nc.sync.dma_start(out_d[:], logits[:])
            return
        # ---------------- epilogue ----------------
        # pooled^T (FM fold [128, 6]) = mean over tokens
        pooled = scr.tile([128, 8], f32, tag="pooled", name="pooled")
        hbf = rbp.tile([128, 2 * C], bf, tag="lnsub", name="hfin")
        nc.vector.tensor_copy(hbf[:], h_t[:])
        for cb in range(6):
            p = psy.tile([128, 512], f32, tag="psy", name="psy")
            for lc in range(2):
                mm(p[:, 0:1], hbf[:, lc * C + cb * 128:lc * C + cb * 128 + 128],
                   onescol[0:128, 0:1], lc == 0, lc == 1)
            nc.vector.tensor_scalar_mul(pooled[:, cb:cb + 1], p[:, 0:1],
                                        1.0 / L)
        # LN over all 768 (partition+fold): stats via f32 matmuls
        sq = scr.tile([128, 8], bf, tag="pooledsq", name="pooledsq")
        sqa = st()
        nc.vector.scalar_tensor_tensor(
            sq[:, 0:6], pooled[:, 0:6], 1.0, pooled[:, 0:6], ALU.mult,
            ALU.mult, accum_out=sqa[:])
        sqab = stp.tile([128, 1], bf, tag="stb", name="stb")
        nc.vector.tensor_copy(sqab[:], sqa[:])
        sma = st()
        nc.vector.reduce_sum(sma[:], pooled[:, 0:6], axis=X_AXIS)
        smab = stp.tile([128, 1], bf, tag="stb", name="stb")
        nc.vector.tensor_copy(smab[:], sma[:])
        pst = psy.tile([128, 512], f32, tag="psy", name="psy")
        mm(pst[0:1, 0:1], smab[:], onescol[0:128, 0:1], True, True)
        mm(pst[0:1, 1:2], sqab[:], onescol[0:128, 0:1], True, True)
        stat2 = zp.tile([1, 2], f32, tag="st2", name="st2")
        nc.vector.tensor_copy(stat2[:], pst[0:1, 0:2])
        mean = zp.tile([1, 2], f32, tag="mv2", name="mv2")
        nc.vector.tensor_scalar_mul(mean[:, 0:1], stat2[:, 0:1], 1.0 / C)
        m2 = zp.tile([1, 1], f32, tag="m2", name="m2")
        nc.vector.tensor_tensor(m2[:], mean[:, 0:1], mean[:, 0:1], ALU.mult)
        var = zp.tile([1, 1], f32, tag="var", name="var")
        nc.vector.scalar_tensor_tensor(var[:], stat2[:, 1:2], 1.0 / C, m2[:],
                                       ALU.mult, ALU.subtract)
        sd = zp.tile([1, 1], f32, tag="sd", name="sd")
        nc.scalar.activation(sd[:], var[:], AF.Sqrt, bias=eps5[0:1, :])
        rr = zp.tile([1, 1], f32, tag="rr", name="rr")
        nc.vector.reciprocal(rr[:], sd[:])
        mrb = zp.tile([2, 2], bf, tag="mrb", name="mrb")
        nc.gpsimd.memset(mrb[:], 0.0)
        nc.vector.tensor_copy(mrb[0:1, 0:1], mean[:, 0:1])
        nc.vector.tensor_copy(mrb[0:1, 1:2], rr[:])
        # broadcast mean, rstd to [128, 1] via K=1 f32... use bf16 matmul
        pbc = psy.tile([128, 512], f32, tag="psy", name="psy")
        mm(pbc[0:128, 0:2], ones2[0:2, 0:128], mrb[0:2, 0:2], True, True)
        mbc = st()
        nc.vector.tensor_copy(mbc[:], pbc[0:128, 0:1])
        rbc = st()
        nc.vector.tensor_copy(rbc[:], pbc[0:128, 1:2])
        pn = scr.tile([128, 8], f32, tag="pn", name="pn")
        nc.vector.tensor_scalar(pn[:, 0:6], pooled[:, 0:6], mbc[:], rbc[:],
                                ALU.subtract, ALU.mult)
        nc.vector.tensor_tensor(pn[:, 0:6], pn[:, 0:6], normgb[:, 0:6],
                                ALU.mult)
        nc.vector.tensor_add(pn[:, 0:6], pn[:, 0:6], normgb[:, 6:12])
        # head (f32 matmuls)
        ph = psy.tile([128, 512], f32, tag="psy", name="psy")
        for cb in range(6):
            mm(ph[0:10, 0:1], headwT[:, cb * 10:(cb + 1) * 10],
               pn[:, cb:cb + 1], cb == 0, cb == 5)
        logits = scr.tile([10, 1], f32, tag="logits", name="logits")
        nc.vector.tensor_add(logits[:], ph[0:10, 0:1], headb[0:10, 0:1])
        nc.sync.dma_start(out_d[:], logits[:])


_NC_CACHE = {}
TRACE = False
LAST = {}


def _get_nc(in_map):
    key = "k"
    if key not in _NC_CACHE:
        _NC_CACHE[key] = build(in_map)
    return _NC_CACHE[key]


def kernel(**inputs):
    per_core = marshal(inputs)
    nc = _get_nc(per_core[0])
    res = run_bass_kernel_spmd(nc, per_core, core_ids=list(range(N_CORES)),
                               trace=TRACE)
    LAST["exec_time_ns"] = res.exec_time_ns
    out = np.stack([res.results[b]["out"][:, 0] for b in range(N_CORES)])
    return out.astype(np.float32)

